# revision 43
# baseline (speedup 1.0000x reference)
"""AttentionAggregator Trainium2 kernel.

Reference (per batch b, head h):
  qh = x_q @ Wq_h^T; kh = x @ Wk_h^T
  attn = softmax(qh @ kh^T / 8)
  heads_h = (attn @ r) @ Wv_h^T == attn @ (r @ Wv_h^T)   (associativity)
  out = concat_h(heads_h) @ Wo^T

Sharding: data-parallel over batch B=16 across 8 cores (2 batches/core).
No collectives. Projections/scores in float32r, attention weights and vh
in bf16 (PE full rate either way; ~1e-3 rel).

Layouts (contractions on partitions):
  xqT/xT/rT  [d=128 x4][n=1024]    PE-transposed input tiles
  qhT/khT    [e=128(2 heads) x4][nq]
  vh         [m=128 x8][8 heads, col 0 = ones -> softmax denom, cols 1-65 v]
  scoresT    psum [m=128, nq=1024]; ACT exp (scale=1/8) -> attnT bf16
  headsT     psum 2x[65, 512] accumulated over m; row 0 = denom
  normalize  DVE recip_approx + PE outer-product broadcast + DVE mul
  out        [nq=128, 512] = concatT^T @ WoT (4 e-chunks)

Schedule: the attention inner loop is paced by the ACT exp (~1.3us/step);
all projections / input transposes / output tiles are queued as small PE
"pieces" and one piece is emitted per attention step BETWEEN the score
and heads matmuls, so the PE never idles and its DVFS clock stays ramped.
Batch-1 work fills batch-0's attention; batch-0's output projection and
batch-1's late head projections fill batch-1's attention.
"""

import sys

sys.path.insert(0, "/opt/trn_rl_repo")

import numpy as np

B, N, NQ, D, H = 16, 1024, 1024, 512, 8
HD = D // H  # 64
P = 128
NCORES = 8
BLOC = B // NCORES
ND = D // P    # 4 d-chunks
NM = N // P    # 8 m-tiles
NNQ = NQ // P  # 8 nq-tiles
FREE = 512

_CACHE = {}


def _build(debug_dump=False):
    import concourse.mybir as mybir
    from concourse.bacc import Bacc
    from concourse.tile import TileContext
    from concourse.masks import make_identity

    f32 = mybir.dt.float32
    f32r = mybir.dt.float32r
    bf16 = mybir.dt.bfloat16
    AF = mybir.ActivationFunctionType

    nc = Bacc("TRN2", target_bir_lowering=False, debug=False)

    x_d = nc.dram_tensor("x", [BLOC, N, D], f32, kind="ExternalInput")
    r_d = nc.dram_tensor("r", [BLOC, N, D], f32, kind="ExternalInput")
    xq_d = nc.dram_tensor("x_q", [BLOC, NQ, D], f32, kind="ExternalInput")
    wq_d = nc.dram_tensor("Wq", [H, HD, D], f32, kind="ExternalInput")
    wk_d = nc.dram_tensor("Wk", [H, HD, D], f32, kind="ExternalInput")
    wv_d = nc.dram_tensor("Wv", [H, HD, D], f32, kind="ExternalInput")
    wo_d = nc.dram_tensor("Wo", [D, D], f32, kind="ExternalInput")
    out_d = nc.dram_tensor("out", [BLOC, NQ, D], f32, kind="ExternalOutput")
    dbg = {}
    if debug_dump:
        for nm, shape, dt in [("dbg_xqT", [P, NQ], f32r), ("dbg_qhT", [P, NQ], f32r),
                              ("dbg_khT", [P, N], f32r), ("dbg_vh", [P, H, 66], bf16),
                              ("dbg_attnT", [P, NQ], bf16),
                              ("dbg_concatT", [P, NQ], f32r),
                              ("dbg_hc", [65, FREE], f32),
                              ("dbg_rec", [1, FREE], f32r)]:
            dbg[nm] = nc.dram_tensor(nm, shape, dt, kind="ExternalOutput")

    with TileContext(nc) as tc:
        with (
            tc.tile_pool(name="const", bufs=1) as constp,
            tc.tile_pool(name="wgt", bufs=1) as wgt,
            tc.tile_pool(name="big", bufs=1) as big,
            tc.tile_pool(name="stage", bufs=8) as stage,
            tc.tile_pool(name="attn", bufs=3) as attnp,
            tc.tile_pool(name="evac", bufs=4) as evacp,
            tc.tile_pool(name="ps1", bufs=2, space="PSUM") as ps1,
            tc.tile_pool(name="ps_sc", bufs=2, space="PSUM") as ps_sc,
            tc.tile_pool(name="ps_hd", bufs=2, space="PSUM") as ps_hd,
        ):
            ident = constp.tile([P, P], f32, name="ident")
            make_identity(nc, ident)
            ones_c = constp.tile([P, H, 2], f32, name="ones_c")
            nc.any.memset(ones_c[:], 1.0)
            ones1f = constp.tile([1, HD], f32, name="ones1f")
            nc.any.memset(ones1f[:], 1.0)
            ones1 = constp.tile([1, HD], f32r, name="ones1")
            nc.any.tensor_copy(ones1[:], ones1f[:])


            def dump(nm, ap):
                if debug_dump:
                    nc.sync.dma_start(out=dbg[nm].ap(), in_=ap)

            # ---------- weights (transposed via PE, grouped evacs) ----------
            def load_transpose_w(dram_rows_ap, tagpfx):
                """dram [512 rows, 512] -> 4 tiles [d=128, rows=512] (f32r)."""
                tiles = [wgt.tile([P, D], f32r, tag=f"{tagpfx}_{k}", name=f"{tagpfx}_{k}")
                         for k in range(ND)]
                nats = []
                for j in range(4):
                    nat = stage.tile([P, D], f32, tag="nat", name="nat")
                    nc.sync.dma_start(out=nat[:], in_=dram_rows_ap[j * P:(j + 1) * P, :])
                    nats.append(nat)
                for k in range(ND):
                    pt = ps1.tile([P, FREE], f32, tag="proj", name="tpw")
                    for j in range(4):
                        nc.tensor.transpose(
                            pt[:, j * P:(j + 1) * P], nats[j][:, k * P:(k + 1) * P], ident[:])
                    nc.any.tensor_copy(tiles[k][:], pt[:])
                return tiles

            # ---------- input transpose units (split into pieces) ----------
            def alloc_T(tagpfx, b):
                # shared tags across batches: batch 1's writes land after
                # batch 0's last reads in the piece schedule
                return [big.tile([P, NQ], f32r, tag=f"{tagpfx}_{k}",
                                 name=f"{tagpfx}_{k}_{b}") for k in range(ND)]

            class Unit:
                """One [512 rows x 512] -> transposed half of an input tensor."""

                def __init__(self, dram_ap, tiles, half):
                    self.dram_ap, self.tiles, self.half = dram_ap, tiles, half
                    self.nats = []

                def loads(self):
                    for i in range(4):
                        nat = stage.tile([P, D], f32, tag="nat", name="nat")
                        i0 = (self.half * 4 + i) * P
                        nc.sync.dma_start(out=nat[:], in_=self.dram_ap[i0:i0 + P, :])
                        self.nats.append(nat)

                def tp(self, k):
                    pt = ps1.tile([P, FREE], f32, tag="proj", name="tpi")
                    for i in range(4):
                        nc.tensor.transpose(
                            pt[:, i * P:(i + 1) * P],
                            self.nats[i][:, k * P:(k + 1) * P], ident[:])
                    nc.vector.tensor_copy(
                        self.tiles[k][:, self.half * FREE:(self.half + 1) * FREE],
                        pt[:])

            def input_units(b):
                xqT = alloc_T("xqT", b)
                xT = alloc_T("xT", b)
                rT = alloc_T("rT", b)
                units = {}
                for nm, dram, tiles in (("xq", xq_d, xqT), ("x", x_d, xT),
                                        ("r", r_d, rT)):
                    for half in range(2):
                        units[(nm, half)] = Unit(dram.ap()[b], tiles, half)
                return units, {"xqT": xqT, "xT": xT, "rT": rT}

            # ---------- projections (emittable in pieces) ----------
            def alloc_proj(b):
                qhT = [big.tile([P, NQ], f32r, tag=f"qhT_{hp}",
                                name=f"qhT_{hp}_{b}") for hp in range(4)]
                khT = [big.tile([P, N], f32r, tag=f"khT_{hp}",
                                name=f"khT_{hp}_{b}") for hp in range(4)]
                # vh tags are per-batch: batch 1's vh is produced mid-way
                # through batch 0's attention, while batch 0's vh is still
                # being read by its heads matmuls until the last step
                vh = [big.tile([P, H, 66], bf16, tag=f"vh_{m}_{b}",
                               name=f"vh_{m}_{b}") for m in range(NM)]
                return qhT, khT, vh

            def proj_one(wT, xt, dst, hp, c):
                """One projection chunk: dst[hp][:, c*512:...] (4 matmuls+evac)."""
                pp = ps1.tile([P, FREE], f32, tag="proj", name="proj")
                for k in range(ND):
                    nc.tensor.matmul(
                        pp[:], wT[k][:, hp * P:(hp + 1) * P],
                        xt[k][:, c * FREE:(c + 1) * FREE],
                        start=(k == 0), stop=(k == ND - 1))
                nc.vector.tensor_copy(dst[hp][:, c * FREE:(c + 1) * FREE], pp[:])

            def proj_vh(tin, vh, m):
                pp = ps1.tile([P, FREE], f32, tag="proj", name="proj")
                for k in range(ND):
                    nc.tensor.matmul(
                        pp[:], tin["rT"][k][:, m * P:(m + 1) * P], wvT[k][:],
                        start=(k == 0), stop=(k == ND - 1))
                # col 64 = ones -> softmax denom accumulates in psum row 64
                nc.vector.tensor_copy(
                    vh[m][:, :, 0:HD], pp[:].rearrange("p (h e) -> p h e", h=H))
                nc.vector.tensor_copy(vh[m][:, :, 64:66], ones_c[:])

            def qk_pieces(tin, qhT, khT, hp):
                return [
                    lambda c=c, w=w, x=x, d=d: proj_one(w, tin[x], d, hp, c)
                    for c in range(2) for w, x, d in
                    ((wqT, "xqT", qhT), (wkT, "xT", khT))]

            def out_tile(b, concatT, t):
                po = ps1.tile([P, D], f32, tag="proj", name="proj")
                for hp in range(4):
                    nc.tensor.matmul(
                        po[:], concatT[hp][:, t * P:(t + 1) * P], woT[hp][:],
                        start=(hp == 0), stop=(hp == 3))
                ot = evacp.tile([P, D], f32, tag="out", name="out", bufs=2)
                nc.vector.tensor_copy(ot[:], po[:])
                nc.sync.dma_start(out=out_d.ap()[b, t * P:(t + 1) * P, :], in_=ot[:])

            # ---------- attention ----------
            def attention(b, qhT, khT, vh, queue, warm=0, dump_attn=False):
                """queue: PE filler thunks, one emitted per step between the
                score and heads matmuls (keeps the PE stream gapless while
                the ACT exp paces the loop). warm: dummy ldweights per step
                to keep the PE p-state ramped when real filler runs thin."""
                concatT = [big.tile([P, NQ], f32r, tag=f"concatT_{hp}_{b}",
                                    name=f"concatT_{hp}_{b}") for hp in range(4)]
                n_steps = H * NM
                pending = {}

                def norm_stage1(h):  # psum evac + reciprocal (DVE)
                    st = pending[h]
                    st["hc"], st["rec"] = [], []
                    for c in range(2):
                        hc = evacp.tile([65, FREE], f32, tag="hcopy", name="hcopy")
                        nc.vector.tensor_copy(hc[:], st["ph"][c][:])
                        # recip_approx misreads inputs not based at partition
                        # 0 — stage the denom row through a partition-0 tile
                        dcp = evacp.tile([1, FREE], f32, tag="dcp", name="dcp",
                                         bufs=2)
                        nc.vector.tensor_copy(dcp[:], st["ph"][c][64:65, :])
                        rec = evacp.tile([1, FREE], f32, tag="rec", name="rec",
                                         bufs=2)
                        nc.vector.reciprocal_approx_fast(rec[:], dcp[:])
                        recr = evacp.tile([1, FREE], f32r, tag="recr",
                                          name="recr", bufs=2)
                        nc.gpsimd.tensor_copy(recr[:], rec[:])
                        if h == 0 and c == 0 and dump_attn:
                            dump("dbg_hc", hc[:])
                            dump("dbg_rec", recr[:])
                        st["hc"].append(hc)
                        st["rec"].append(recr)

                def norm_stage2(h):  # broadcast 1/denom across partitions (PE)
                    st = pending[h]
                    st["bcp"] = []
                    for c in range(2):
                        bcp = ps1.tile([HD, FREE], f32, tag="proj", name="bcp")
                        nc.tensor.matmul(
                            bcp[:], ones1[:], st["rec"][c][:],
                            start=True, stop=True)
                        st["bcp"].append(bcp)

                def norm_stage3(h):  # normalize into concatT (DVE)
                    st = pending.pop(h)
                    hp, off = h // 2, (h % 2) * HD
                    for c in range(2):
                        nc.vector.tensor_mul(
                            concatT[hp][off:off + HD, c * FREE:(c + 1) * FREE],
                            st["hc"][c][0:HD, :], st["bcp"][c][:])

                def score_mm(j):
                    h, m = divmod(j, NM)
                    hp, off = h // 2, (h % 2) * HD
                    psc = ps_sc.tile([P, NQ], f32, tag="score", name="score")
                    for c in range(NQ // FREE):
                        nc.tensor.matmul(
                            psc[:, c * FREE:(c + 1) * FREE],
                            khT[hp][off:off + HD, m * P:(m + 1) * P],
                            qhT[hp][off:off + HD, c * FREE:(c + 1) * FREE],
                            start=True, stop=True)
                    return psc

                ph = None
                psc_cur = score_mm(0)
                for j in range(n_steps):
                    h, m = divmod(j, NM)
                    if m == 0:
                        if h > 0:
                            # must run before the ph realloc below: it reads
                            # head h-1's psum and frees the heads slots
                            norm_stage1(h - 1)
                        ph = [ps_hd.tile([65, FREE], f32, tag="heads",
                                         name=f"heads{c}") for c in range(2)]
                    at = attnp.tile([P, NQ], bf16, tag="attnT", name="attnT")
                    nc.scalar.activation(at[:], psc_cur[:], AF.Exp, scale=0.125)
                    if dump_attn and j == 0:
                        dump("dbg_attnT", at[:])
                    if j + 1 < n_steps:
                        psc_cur = score_mm(j + 1)
                    if queue:
                        th = queue.pop(0)
                        if th is not None:
                            th()
                    for c in range(2):
                        nc.tensor.matmul(
                            ph[c][:], vh[m][:, h, 0:65],
                            at[:, c * FREE:(c + 1) * FREE],
                            start=(m == 0), stop=(m == NM - 1))
                    if h > 0:
                        if m == 4:
                            norm_stage2(h - 1)
                        elif m == 5:
                            norm_stage3(h - 1)
                    if m == NM - 1:
                        pending[h] = {"ph": ph}
                norm_stage1(H - 1)
                norm_stage2(H - 1)
                norm_stage3(H - 1)
                return concatT

            # ---------- schedule ----------
            wqT = load_transpose_w(wq_d.ap().rearrange("h e d -> (h e) d"), "wqT")
            wkT = load_transpose_w(wk_d.ap().rearrange("h e d -> (h e) d"), "wkT")
            wvT = load_transpose_w(wv_d.ap().rearrange("h e d -> (h e) d"), "wvT")
            woT = load_transpose_w(wo_d.ap(), "woT")

            units0, tin0 = input_units(0)
            q0, k0, v0 = alloc_proj(0)
            units1, tin1 = input_units(1)
            q1, k1, v1 = alloc_proj(1)

            # preamble: batch-0 inputs + its hp0/vh projections (PE-solid)
            A, Bu = units0[("xq", 0)], units0[("x", 0)]
            C, Du = units0[("xq", 1)], units0[("x", 1)]
            E, Fu = units0[("r", 0)], units0[("r", 1)]
            A.loads(); Bu.loads()
            for k in range(4):
                A.tp(k)
            C.loads()
            for k in range(4):
                Bu.tp(k)
            Du.loads()
            proj_one(wqT, tin0["xqT"], q0, 0, 0)
            proj_one(wkT, tin0["xT"], k0, 0, 0)
            for k in range(4):
                C.tp(k)
            E.loads()
            for k in range(4):
                Du.tp(k)
            Fu.loads()
            proj_one(wqT, tin0["xqT"], q0, 0, 1)
            proj_one(wkT, tin0["xT"], k0, 0, 1)
            dump("dbg_xqT", tin0["xqT"][0][:])
            dump("dbg_qhT", q0[0][:])
            dump("dbg_khT", k0[0][:])
            for k in range(4):
                E.tp(k)
            for m in range(4):
                proj_vh(tin0, v0, m)
            for k in range(4):
                Fu.tp(k)
            for m in range(4, NM):
                proj_vh(tin0, v0, m)
            dump("dbg_vh", v0[0][:])

            # queue for batch-0 attention: batch-0 late head projections,
            # then all batch-1 transposes + its hp0/vh projections
            def unit_pieces(seq):
                """Pieces for a run of units; unit i+1's DMA loads piggyback
                on unit i's k=1 piece (~3 steps of lead time)."""
                pieces = []
                for ui, u in enumerate(seq):
                    if ui + 1 < len(seq):
                        nxt = seq[ui + 1]
                        pieces += [lambda u=u: u.tp(0),
                                   lambda u=u, nx=nxt: (u.tp(1), nx.loads()),
                                   lambda u=u: u.tp(2), lambda u=u: u.tp(3)]
                    else:
                        pieces += [lambda u=u, k=k: u.tp(k) for k in range(4)]
                return pieces

            useq = [units1[("xq", 0)], units1[("xq", 1)], units1[("x", 0)],
                    units1[("x", 1)], units1[("r", 0)], units1[("r", 1)]]
            units1[("xq", 0)].loads()  # prefetch first unit's DMA now
            queue0 = []
            for hp in range(1, 4):
                queue0 += qk_pieces(tin0, q0, k0, hp)
            queue0 += unit_pieces(useq)
            queue0 += qk_pieces(tin1, q1, k1, 0)[:2]
            queue0 += [lambda m=m: proj_vh(tin1, v1, m) for m in range(NM)]
            queue0 += qk_pieces(tin1, q1, k1, 0)[2:]

            c0 = attention(0, q0, k0, v0, queue0, dump_attn=True)
            dump("dbg_concatT", c0[0][:])

            # queue for batch-1 attention: its late head projections + the
            # batch-0 output tiles
            queue1 = []
            for hp in range(1, 4):
                for p in qk_pieces(tin1, q1, k1, hp):
                    queue1 += [p, None]
            for t in range(NNQ):
                queue1 += [lambda t=t: out_tile(0, c0, t), None, None]

            c1 = attention(1, q1, k1, v1, queue1, warm=3)
            for t in range(NNQ):
                out_tile(1, c1, t)

    nc.finalize()
    return nc


def _get_nc():
    if "nc" not in _CACHE:
        _CACHE["nc"] = _build()
    return _CACHE["nc"]


def kernel(x, r, x_q, Wq, Wk, Wv, Wo, **kw):
    from concourse.bass_utils import run_bass_kernel_spmd

    nc = _get_nc()
    x = np.ascontiguousarray(x, np.float32)
    r = np.ascontiguousarray(r, np.float32)
    x_q = np.ascontiguousarray(x_q, np.float32)
    in_maps = []
    for c in range(NCORES):
        sl = slice(c * BLOC, (c + 1) * BLOC)
        in_maps.append({
            "x": x[sl], "r": r[sl], "x_q": x_q[sl],
            "Wq": np.ascontiguousarray(Wq, np.float32),
            "Wk": np.ascontiguousarray(Wk, np.float32),
            "Wv": np.ascontiguousarray(Wv, np.float32),
            "Wo": np.ascontiguousarray(Wo, np.float32),
        })
    res = run_bass_kernel_spmd(nc, in_maps, list(range(NCORES)), **kw)
    out = np.concatenate([res.results[c]["out"] for c in range(NCORES)], axis=0)
    _CACHE["last_results"] = res
    return out


# revision 44
# speedup vs baseline: 1.1993x; 1.1993x over previous
"""AttentionAggregator Trainium2 kernel.

Reference (per batch b, head h):
  qh = x_q @ Wq_h^T; kh = x @ Wk_h^T
  attn = softmax(qh @ kh^T / 8)
  heads_h = (attn @ r) @ Wv_h^T == attn @ (r @ Wv_h^T)   (associativity)
  out = concat_h(heads_h) @ Wo^T

Sharding: data-parallel over batch B=16 across 8 cores (2 batches/core).
No collectives. Projections/scores in float32r, attention weights and vh
in bf16 (PE full rate either way; ~1e-3 rel).

Layouts (contractions on partitions):
  xqT/xT/rT  [d=128 x4][n=1024]    PE-transposed input tiles
  qhT/khT    [e=128(2 heads) x4][nq]
  vh         [m=128 x8][8 heads, col 0 = ones -> softmax denom, cols 1-65 v]
  scoresT    psum [m=128, nq=1024]; ACT exp (scale=1/8) -> attnT bf16
  headsT     psum 2x[65, 512] accumulated over m; row 0 = denom
  normalize  DVE recip_approx + PE outer-product broadcast + DVE mul
  out        [nq=128, 512] = concatT^T @ WoT (4 e-chunks)

Schedule: the attention inner loop is paced by the ACT exp (~1.3us/step);
all projections / input transposes / output tiles are queued as small PE
"pieces" and one piece is emitted per attention step BETWEEN the score
and heads matmuls, so the PE never idles and its DVFS clock stays ramped.
Batch-1 work fills batch-0's attention; batch-0's output projection and
batch-1's late head projections fill batch-1's attention.
"""

import sys

sys.path.insert(0, "/opt/trn_rl_repo")

import numpy as np

B, N, NQ, D, H = 16, 1024, 1024, 512, 8
HD = D // H  # 64
P = 128
NCORES = 8
BLOC = B // NCORES
ND = D // P    # 4 d-chunks
NM = N // P    # 8 m-tiles
NNQ = NQ // P  # 8 nq-tiles
FREE = 512

_CACHE = {}


def _build(debug_dump=False):
    import concourse.mybir as mybir
    from concourse.bacc import Bacc
    from concourse.tile import TileContext
    from concourse.masks import make_identity

    f32 = mybir.dt.float32
    f32r = mybir.dt.float32r
    bf16 = mybir.dt.bfloat16
    AF = mybir.ActivationFunctionType

    nc = Bacc("TRN2", target_bir_lowering=False, debug=False)

    x_d = nc.dram_tensor("x", [BLOC, N, D], f32, kind="ExternalInput")
    r_d = nc.dram_tensor("r", [BLOC, N, D], f32, kind="ExternalInput")
    xq_d = nc.dram_tensor("x_q", [BLOC, NQ, D], f32, kind="ExternalInput")
    wq_d = nc.dram_tensor("Wq", [H, HD, D], f32, kind="ExternalInput")
    wk_d = nc.dram_tensor("Wk", [H, HD, D], f32, kind="ExternalInput")
    wv_d = nc.dram_tensor("Wv", [H, HD, D], f32, kind="ExternalInput")
    wo_d = nc.dram_tensor("Wo", [D, D], f32, kind="ExternalInput")
    out_d = nc.dram_tensor("out", [BLOC, NQ, D], f32, kind="ExternalOutput")
    dbg = {}
    if debug_dump:
        for nm, shape, dt in [("dbg_xqT", [P, NQ], f32r), ("dbg_qhT", [P, NQ], f32r),
                              ("dbg_khT", [P, N], f32r), ("dbg_vh", [P, H, 66], bf16),
                              ("dbg_attnT", [P, NQ], bf16),
                              ("dbg_concatT", [P, NQ], f32r),
                              ("dbg_hc", [65, FREE], f32),
                              ("dbg_rec", [1, FREE], f32r)]:
            dbg[nm] = nc.dram_tensor(nm, shape, dt, kind="ExternalOutput")

    with TileContext(nc) as tc:
        with (
            tc.tile_pool(name="const", bufs=1) as constp,
            tc.tile_pool(name="wgt", bufs=1) as wgt,
            tc.tile_pool(name="big", bufs=1) as big,
            tc.tile_pool(name="stage", bufs=8) as stage,
            tc.tile_pool(name="attn", bufs=3) as attnp,
            tc.tile_pool(name="evac", bufs=4) as evacp,
            tc.tile_pool(name="ps1", bufs=2, space="PSUM") as ps1,
            tc.tile_pool(name="ps_sc", bufs=2, space="PSUM") as ps_sc,
            tc.tile_pool(name="ps_hd", bufs=2, space="PSUM") as ps_hd,
        ):
            ident = constp.tile([P, P], f32, name="ident")
            make_identity(nc, ident)
            ones_c = constp.tile([P, H, 2], f32, name="ones_c")
            nc.any.memset(ones_c[:], 1.0)
            ones1f = constp.tile([1, HD], f32, name="ones1f")
            nc.any.memset(ones1f[:], 1.0)
            ones1 = constp.tile([1, HD], f32r, name="ones1")
            nc.any.tensor_copy(ones1[:], ones1f[:])


            def dump(nm, ap):
                if debug_dump:
                    nc.sync.dma_start(out=dbg[nm].ap(), in_=ap)

            # ---------- weights (transposed via PE, grouped evacs) ----------
            def load_transpose_w(dram_rows_ap, tagpfx):
                """dram [512 rows, 512] -> 4 tiles [d=128, rows=512] (f32r)."""
                tiles = [wgt.tile([P, D], f32r, tag=f"{tagpfx}_{k}", name=f"{tagpfx}_{k}")
                         for k in range(ND)]
                nats = []
                for j in range(4):
                    nat = stage.tile([P, D], f32, tag="nat", name="nat")
                    nc.sync.dma_start(out=nat[:], in_=dram_rows_ap[j * P:(j + 1) * P, :])
                    nats.append(nat)
                for k in range(ND):
                    pt = ps1.tile([P, FREE], f32, tag="proj", name="tpw")
                    for j in range(4):
                        nc.tensor.transpose(
                            pt[:, j * P:(j + 1) * P], nats[j][:, k * P:(k + 1) * P], ident[:])
                    nc.any.tensor_copy(tiles[k][:], pt[:])
                return tiles

            # ---------- input transpose units (split into pieces) ----------
            def alloc_T(tagpfx, b):
                # shared tags across batches: batch 1's writes land after
                # batch 0's last reads in the piece schedule
                return [big.tile([P, NQ], f32r, tag=f"{tagpfx}_{k}",
                                 name=f"{tagpfx}_{k}_{b}") for k in range(ND)]

            class Unit:
                """One [512 rows x 512] -> transposed half of an input tensor."""

                def __init__(self, dram_ap, tiles, half):
                    self.dram_ap, self.tiles, self.half = dram_ap, tiles, half
                    self.nats = []

                def loads(self):
                    for i in range(4):
                        nat = stage.tile([P, D], f32, tag="nat", name="nat")
                        i0 = (self.half * 4 + i) * P
                        nc.sync.dma_start(out=nat[:], in_=self.dram_ap[i0:i0 + P, :])
                        self.nats.append(nat)

                def tp(self, k):
                    pt = ps1.tile([P, FREE], f32, tag="proj", name="tpi")
                    for i in range(4):
                        nc.tensor.transpose(
                            pt[:, i * P:(i + 1) * P],
                            self.nats[i][:, k * P:(k + 1) * P], ident[:])
                    nc.vector.tensor_copy(
                        self.tiles[k][:, self.half * FREE:(self.half + 1) * FREE],
                        pt[:])

            def input_units(b):
                xqT = alloc_T("xqT", b)
                xT = alloc_T("xT", b)
                rT = alloc_T("rT", b)
                units = {}
                for nm, dram, tiles in (("xq", xq_d, xqT), ("x", x_d, xT),
                                        ("r", r_d, rT)):
                    for half in range(2):
                        units[(nm, half)] = Unit(dram.ap()[b], tiles, half)
                return units, {"xqT": xqT, "xT": xT, "rT": rT}

            # ---------- projections (emittable in pieces) ----------
            def alloc_proj(b):
                qhT = [big.tile([P, NQ], f32r, tag=f"qhT_{hp}",
                                name=f"qhT_{hp}_{b}") for hp in range(4)]
                khT = [big.tile([P, N], f32r, tag=f"khT_{hp}",
                                name=f"khT_{hp}_{b}") for hp in range(4)]
                # vh tags are per-batch: batch 1's vh is produced mid-way
                # through batch 0's attention, while batch 0's vh is still
                # being read by its heads matmuls until the last step
                vh = [big.tile([P, H, 66], bf16, tag=f"vh_{m}_{b}",
                               name=f"vh_{m}_{b}") for m in range(NM)]
                return qhT, khT, vh

            def proj_one(wT, xt, dst, hp, c):
                """One projection chunk: dst[hp][:, c*512:...] (4 matmuls+evac)."""
                pp = ps1.tile([P, FREE], f32, tag="proj", name="proj")
                for k in range(ND):
                    nc.tensor.matmul(
                        pp[:], wT[k][:, hp * P:(hp + 1) * P],
                        xt[k][:, c * FREE:(c + 1) * FREE],
                        start=(k == 0), stop=(k == ND - 1))
                nc.vector.tensor_copy(dst[hp][:, c * FREE:(c + 1) * FREE], pp[:])

            def proj_vh(tin, vh, m):
                pp = ps1.tile([P, FREE], f32, tag="proj", name="proj")
                for k in range(ND):
                    nc.tensor.matmul(
                        pp[:], tin["rT"][k][:, m * P:(m + 1) * P], wvT[k][:],
                        start=(k == 0), stop=(k == ND - 1))
                # col 64 = ones -> softmax denom accumulates in psum row 64
                nc.vector.tensor_copy(
                    vh[m][:, :, 0:HD], pp[:].rearrange("p (h e) -> p h e", h=H))
                nc.vector.tensor_copy(vh[m][:, :, 64:66], ones_c[:])

            def qk_pieces(tin, qhT, khT, hp):
                return [
                    lambda c=c, w=w, x=x, d=d: proj_one(w, tin[x], d, hp, c)
                    for c in range(2) for w, x, d in
                    ((wqT, "xqT", qhT), (wkT, "xT", khT))]

            def out_tile(b, concatT, t):
                po = ps1.tile([P, D], f32, tag="proj", name="proj")
                for hp in range(4):
                    nc.tensor.matmul(
                        po[:], concatT[hp][:, t * P:(t + 1) * P], woT[hp][:],
                        start=(hp == 0), stop=(hp == 3))
                ot = evacp.tile([P, D], f32, tag="out", name="out", bufs=2)
                nc.vector.tensor_copy(ot[:], po[:])
                nc.sync.dma_start(out=out_d.ap()[b, t * P:(t + 1) * P, :], in_=ot[:])

            # ---------- attention ----------
            def attention(b, qhT, khT, vh, queue, warm=0, dump_attn=False):
                """queue: PE filler thunks, one emitted per step between the
                score and heads matmuls (keeps the PE stream gapless while
                the ACT exp paces the loop). warm: dummy ldweights per step
                to keep the PE p-state ramped when real filler runs thin."""
                concatT = [big.tile([P, NQ], f32r, tag=f"concatT_{hp}_{b}",
                                    name=f"concatT_{hp}_{b}") for hp in range(4)]
                n_steps = H * NM
                pending = {}

                def norm_stage1(h):  # psum evac + reciprocal (DVE)
                    st = pending[h]
                    st["hc"], st["rec"] = [], []
                    for c in range(2):
                        hc = evacp.tile([65, FREE], f32, tag="hcopy", name="hcopy")
                        nc.vector.tensor_copy(hc[:], st["ph"][c][:])
                        # recip_approx misreads inputs not based at partition
                        # 0 — stage the denom row through a partition-0 tile
                        dcp = evacp.tile([1, FREE], f32, tag="dcp", name="dcp",
                                         bufs=2)
                        nc.vector.tensor_copy(dcp[:], st["ph"][c][64:65, :])
                        rec = evacp.tile([1, FREE], f32, tag="rec", name="rec",
                                         bufs=2)
                        nc.vector.reciprocal_approx_fast(rec[:], dcp[:])
                        recr = evacp.tile([1, FREE], f32r, tag="recr",
                                          name="recr", bufs=2)
                        nc.gpsimd.tensor_copy(recr[:], rec[:])
                        if h == 0 and c == 0 and dump_attn:
                            dump("dbg_hc", hc[:])
                            dump("dbg_rec", recr[:])
                        st["hc"].append(hc)
                        st["rec"].append(recr)

                def norm_stage2(h):  # broadcast 1/denom across partitions (PE)
                    st = pending[h]
                    st["bcp"] = []
                    for c in range(2):
                        bcp = ps1.tile([HD, FREE], f32, tag="proj", name="bcp")
                        nc.tensor.matmul(
                            bcp[:], ones1[:], st["rec"][c][:],
                            start=True, stop=True)
                        st["bcp"].append(bcp)

                def norm_stage3(h):  # normalize into concatT (DVE)
                    st = pending.pop(h)
                    hp, off = h // 2, (h % 2) * HD
                    for c in range(2):
                        nc.vector.tensor_mul(
                            concatT[hp][off:off + HD, c * FREE:(c + 1) * FREE],
                            st["hc"][c][0:HD, :], st["bcp"][c][:])

                def score_mm(j):
                    h, m = divmod(j, NM)
                    hp, off = h // 2, (h % 2) * HD
                    psc = ps_sc.tile([P, NQ], f32, tag="score", name="score")
                    for c in range(NQ // FREE):
                        nc.tensor.matmul(
                            psc[:, c * FREE:(c + 1) * FREE],
                            khT[hp][off:off + HD, m * P:(m + 1) * P],
                            qhT[hp][off:off + HD, c * FREE:(c + 1) * FREE],
                            start=True, stop=True)
                    return psc

                ph = None
                psc_cur = score_mm(0)
                for j in range(n_steps):
                    h, m = divmod(j, NM)
                    if m == 0:
                        if h > 0:
                            # must run before the ph realloc below: it reads
                            # head h-1's psum and frees the heads slots
                            norm_stage1(h - 1)
                        ph = [ps_hd.tile([65, FREE], f32, tag="heads",
                                         name=f"heads{c}") for c in range(2)]
                    at = attnp.tile([P, NQ], bf16, tag="attnT", name="attnT")
                    nc.scalar.activation(at[:], psc_cur[:], AF.Exp, scale=0.125)
                    if dump_attn and j == 0:
                        dump("dbg_attnT", at[:])
                    if j + 1 < n_steps:
                        psc_cur = score_mm(j + 1)
                    if queue:
                        th = queue.pop(0)
                        if th is not None:
                            th()
                    for c in range(2):
                        nc.tensor.matmul(
                            ph[c][:], vh[m][:, h, 0:65],
                            at[:, c * FREE:(c + 1) * FREE],
                            start=(m == 0), stop=(m == NM - 1))
                    if h > 0:
                        if m == 4:
                            norm_stage2(h - 1)
                        elif m == 5:
                            norm_stage3(h - 1)
                    if m == NM - 1:
                        pending[h] = {"ph": ph}
                norm_stage1(H - 1)
                norm_stage2(H - 1)
                norm_stage3(H - 1)
                return concatT

            # ---------- schedule ----------
            wqT = load_transpose_w(wq_d.ap().rearrange("h e d -> (h e) d"), "wqT")
            wkT = load_transpose_w(wk_d.ap().rearrange("h e d -> (h e) d"), "wkT")
            wvT = load_transpose_w(wv_d.ap().rearrange("h e d -> (h e) d"), "wvT")
            woT = load_transpose_w(wo_d.ap(), "woT")

            units0, tin0 = input_units(0)
            q0, k0, v0 = alloc_proj(0)
            units1, tin1 = input_units(1)
            q1, k1, v1 = alloc_proj(1)

            # preamble: batch-0 inputs + its hp0/vh projections (PE-solid)
            A, Bu = units0[("xq", 0)], units0[("x", 0)]
            C, Du = units0[("xq", 1)], units0[("x", 1)]
            E, Fu = units0[("r", 0)], units0[("r", 1)]
            A.loads(); Bu.loads()
            for k in range(4):
                A.tp(k)
            C.loads()
            for k in range(4):
                Bu.tp(k)
            Du.loads()
            proj_one(wqT, tin0["xqT"], q0, 0, 0)
            proj_one(wkT, tin0["xT"], k0, 0, 0)
            for k in range(4):
                C.tp(k)
            E.loads()
            for k in range(4):
                Du.tp(k)
            Fu.loads()
            proj_one(wqT, tin0["xqT"], q0, 0, 1)
            proj_one(wkT, tin0["xT"], k0, 0, 1)
            dump("dbg_xqT", tin0["xqT"][0][:])
            dump("dbg_qhT", q0[0][:])
            dump("dbg_khT", k0[0][:])
            for k in range(4):
                E.tp(k)
            for m in range(4):
                proj_vh(tin0, v0, m)
            for k in range(4):
                Fu.tp(k)
            for m in range(4, NM):
                proj_vh(tin0, v0, m)
            dump("dbg_vh", v0[0][:])

            # queue for batch-0 attention: batch-0 late head projections,
            # then all batch-1 transposes + its hp0/vh projections
            def unit_pieces(seq):
                """Pieces for a run of units; unit i+1's DMA loads piggyback
                on unit i's k=1 piece (~3 steps of lead time)."""
                pieces = []
                for ui, u in enumerate(seq):
                    if ui + 1 < len(seq):
                        nxt = seq[ui + 1]
                        pieces += [lambda u=u: u.tp(0),
                                   lambda u=u, nx=nxt: (u.tp(1), nx.loads()),
                                   lambda u=u: u.tp(2), lambda u=u: u.tp(3)]
                    else:
                        pieces += [lambda u=u, k=k: u.tp(k) for k in range(4)]
                return pieces

            useq = [units1[("xq", 0)], units1[("xq", 1)], units1[("x", 0)],
                    units1[("x", 1)], units1[("r", 0)], units1[("r", 1)]]
            units1[("xq", 0)].loads()  # prefetch first unit's DMA now
            queue0 = []
            for hp in range(1, 4):
                queue0 += qk_pieces(tin0, q0, k0, hp)
            queue0 += unit_pieces(useq)
            queue0 += qk_pieces(tin1, q1, k1, 0)[:2]
            queue0 += [lambda m=m: proj_vh(tin1, v1, m) for m in range(NM)]
            queue0 += qk_pieces(tin1, q1, k1, 0)[2:]

            c0 = attention(0, q0, k0, v0, queue0, dump_attn=True)
            dump("dbg_concatT", c0[0][:])

            # queue for batch-1 attention: its late head projections + the
            # batch-0 output tiles
            queue1 = []
            for hp in range(1, 4):
                queue1 += qk_pieces(tin1, q1, k1, hp)
            queue1 += [lambda t=t: out_tile(0, c0, t) for t in range(NNQ)]

            c1 = attention(1, q1, k1, v1, queue1, warm=3)
            for t in range(NNQ):
                out_tile(1, c1, t)

    nc.finalize()
    return nc


def _get_nc():
    if "nc" not in _CACHE:
        _CACHE["nc"] = _build()
    return _CACHE["nc"]


def kernel(x, r, x_q, Wq, Wk, Wv, Wo, **kw):
    from concourse.bass_utils import run_bass_kernel_spmd

    nc = _get_nc()
    x = np.ascontiguousarray(x, np.float32)
    r = np.ascontiguousarray(r, np.float32)
    x_q = np.ascontiguousarray(x_q, np.float32)
    in_maps = []
    for c in range(NCORES):
        sl = slice(c * BLOC, (c + 1) * BLOC)
        in_maps.append({
            "x": x[sl], "r": r[sl], "x_q": x_q[sl],
            "Wq": np.ascontiguousarray(Wq, np.float32),
            "Wk": np.ascontiguousarray(Wk, np.float32),
            "Wv": np.ascontiguousarray(Wv, np.float32),
            "Wo": np.ascontiguousarray(Wo, np.float32),
        })
    res = run_bass_kernel_spmd(nc, in_maps, list(range(NCORES)), **kw)
    out = np.concatenate([res.results[c]["out"] for c in range(NCORES)], axis=0)
    _CACHE["last_results"] = res
    return out
